# revision 2
# baseline (speedup 1.0000x reference)
"""GCN (2-layer GraphConv) Trainium2 kernel, 8-core SPMD, paired-descriptor
gather.

Each layer: z = spmm(A, table), h = relu(z @ W) on core-local 128-row
blocks; spmm is the one-hot matmul (DVE builds st[t,r] = val_t*(row_t==r),
PE accumulates z^T[feat,row] += msgs^T @ st in PSUM).

DMA scheme vs the 1-row-per-descriptor baseline: gather descriptors fetch
TWO consecutive table rows (elem_size=256 fp16 = 512B, elem_step=128) from a
per-core per-chunk PERMUTED table, so one descriptor serves two edge tokens
whenever their cols sit adjacent in the permutation.  A host-side greedy
builds, per (core, chunk), a linear forest of col pairs (degree<=2, acyclic,
pairs co-occurring in a cell) realized as the permutation; ~74% of tokens
pair, cutting gather descriptors (the dominant modeled cost) ~1.6x.
Unmatched tokens cluster per call after the paired region, so their dead
B-halves form fully-dead slabs that cost no DVE/PE work.  Group-PSUM
accumulation (4 row-blocks = one 2KB bank, one dummy start matmul zeroing
the bank) lets slabs spanning two cells use a single wide st instead of two
instances; st tiles live in per-call arenas so the tile framework emits one
DVE wait per arena instead of one per slab.
"""

import hashlib
import numpy as np
from contextlib import ExitStack

import ml_dtypes
import concourse.bass as bass
import concourse.tile as tile
from concourse import bacc, mybir
from concourse.bass_utils import run_bass_kernel_spmd

f16 = np.float16
dt = mybir.dt

N_NODES = 100000
D = 128
NCORES = 8
NCHUNKS = 4
CHUNK = 25000
CHUNKP = 25008          # chunk rows incl. zero pad
NRB = 98
RPC = NRB * 128
GROUP = 8               # row-blocks per PSUM group (two 2KB banks)

LAST_EXEC_NS = None


# ---------------------------------------------------------------------------
# host-side preprocessing
# ---------------------------------------------------------------------------

def _balance_rows(rows, cols):
    """Deal nodes to cores (degree snake), then greedy-pack each core's
    nodes into NRB buckets minimizing max per-chunk cell load."""
    chunk = cols // CHUNK
    d4 = np.zeros((N_NODES, NCHUNKS), np.int32)
    np.add.at(d4, (rows, chunk), 1)
    dtot = d4.sum(1)

    order = np.argsort(-dtot, kind="stable")
    core_of = np.empty(N_NODES, np.int32)
    pos = 0
    rnd = 0
    while pos < N_NODES:
        take = min(NCORES, N_NODES - pos)
        sl = order[pos:pos + take]
        if rnd % 2 == 0:
            core_of[sl] = np.arange(take)
        else:
            core_of[sl] = NCORES - 1 - np.arange(take)
        pos += take
        rnd += 1

    rb_of = np.empty(N_NODES, np.int32)
    r_of = np.empty(N_NODES, np.int32)
    node_of = np.full((NCORES, NRB, 128), -1, np.int64)
    cellcnt = np.zeros((NCORES, NRB, NCHUNKS), np.int64)
    for k in range(NCORES):
        nodes_k = order[core_of[order] == k]     # degree-desc
        cnt = np.zeros((NRB, NCHUNKS), np.int64)
        nrows = np.zeros(NRB, np.int64)
        for n in nodes_k:
            newmax = (cnt + d4[n][None, :]).max(1)
            newmax[nrows >= 128] = 1 << 40
            b = int(np.argmin(newmax * 100000 + cnt.sum(1) + nrows))
            rb_of[n] = b
            r_of[n] = nrows[b]
            node_of[k, b, nrows[b]] = n
            cnt[b] += d4[n]
            nrows[b] += 1
        cellcnt[k] = cnt

    try:
        from scipy.optimize import linear_sum_assignment
    except ImportError:
        linear_sum_assignment = None
    perm = np.empty((NCORES, NRB), np.int64)
    ranked0 = np.argsort(-cellcnt[0].sum(1), kind="stable")
    perm[0][ranked0] = np.arange(NRB)
    Bcur = cellcnt[0][ranked0].astype(np.float64)
    for k in range(1, NCORES):
        if linear_sum_assignment is None:
            ranked = np.argsort(-cellcnt[k].sum(1), kind="stable")
            perm[k][ranked] = np.arange(NRB)
            Bcur = np.maximum(Bcur, cellcnt[k][ranked])
            continue
        cost = np.maximum(
            cellcnt[k][:, None, :] - Bcur[None, :, :], 0).sum(2)
        ri, cj = linear_sum_assignment(cost)
        perm[k][ri] = cj
        Bcur = np.maximum(Bcur, cellcnt[k][ri][np.argsort(cj)])
    rb_new = perm[core_of, rb_of]
    node_of2 = np.full((NCORES, NRB, 128), -1, np.int64)
    for k in range(NCORES):
        node_of2[k, perm[k]] = node_of[k]
    return core_of, rb_new, r_of, node_of2


def _pair_core_chunk(cell_of_tok, col_of_tok):
    """Greedy pairing for one (core, chunk): pair tokens within a cell so the
    distinct col-pair graph is a linear forest (degree<=2, acyclic).

    Returns (pairs, unmatched, nbrs):
      pairs[rb]: arrays (colA, tokA, colB, tokB)
      unmatched[rb]: arrays (col, tok)
      nbrs: dict col -> [partner cols]  (the forest)
    """
    order = np.argsort(cell_of_tok, kind="stable")
    cells = cell_of_tok[order]
    starts = np.searchsorted(cells, np.arange(NRB))
    ends = np.searchsorted(cells, np.arange(NRB), side="right")

    deg = np.zeros(CHUNK, np.int8)
    parent = np.arange(CHUNK, dtype=np.int32)

    def find(x):
        while parent[x] != x:
            parent[x] = parent[parent[x]]
            x = parent[x]
        return x

    nbrs = {}
    pairs = []
    unmatched = []
    for b in range(NRB):
        tidx = order[starts[b]:ends[b]]
        toks = col_of_tok[tidx]
        pb = []
        ub = []
        if len(toks):
            incell = {}
            for c_, t_ in zip(toks.tolist(), tidx.tolist()):
                incell.setdefault(c_, []).append(t_)
            # reuse existing forest adjacencies first
            for a in list(incell.keys()):
                la = incell.get(a)
                if not la:
                    continue
                for bb in nbrs.get(a, ()):
                    lb = incell.get(bb)
                    while la and lb:
                        pb.append((a, la.pop(), bb, lb.pop()))
            rem = [(a, t_) for a, lst in incell.items() for t_ in lst]
            rem.sort(key=lambda x: -deg[x[0]])
            i = 0
            while i < len(rem):
                a, ta = rem[i]
                i += 1
                if ta is None:
                    continue
                if deg[a] >= 2:
                    ub.append((a, ta))
                    continue
                found = False
                for j in range(i, len(rem)):
                    bcol, tb = rem[j]
                    if tb is None or bcol == a or deg[bcol] >= 2:
                        continue
                    if find(a) != find(bcol):
                        rem[j] = (bcol, None)
                        deg[a] += 1
                        deg[bcol] += 1
                        parent[find(a)] = find(bcol)
                        nbrs.setdefault(a, []).append(bcol)
                        nbrs.setdefault(bcol, []).append(a)
                        pb.append((a, ta, bcol, tb))
                        found = True
                        break
                if not found:
                    ub.append((a, ta))
        pairs.append(np.array(pb, np.int64).reshape(-1, 4))
        unmatched.append(np.array(ub, np.int64).reshape(-1, 2))
    return pairs, unmatched, nbrs


def _forest_to_positions(nbrs):
    """Lay the linear forest out as a sequence; return pos[col]."""
    pos = np.full(CHUNK, -1, np.int32)
    cur = 0
    visited = np.zeros(CHUNK, bool)
    for start, ns in nbrs.items():
        if visited[start] or len(ns) != 1:
            continue
        prev = -1
        node = start
        while True:
            pos[node] = cur
            cur += 1
            visited[node] = True
            nxt = -1
            for nb in nbrs.get(node, ()):
                if nb != prev and not visited[nb]:
                    nxt = nb
                    break
            if nxt < 0:
                break
            prev = node
            node = nxt
    for a in range(CHUNK):
        if pos[a] < 0:
            pos[a] = cur
            cur += 1
    assert cur == CHUNK
    return pos


def prep_edges(adj_rows, adj_cols, adj_vals):
    rows = np.asarray(adj_rows).astype(np.int64)
    cols = np.asarray(adj_cols).astype(np.int64)
    vals = np.asarray(adj_vals).astype(np.float32)

    core_of, rb_of, r_of, node_of = _balance_rows(rows, cols)

    chunk = cols // CHUNK
    ecore = core_of[rows]
    erb = rb_of[rows]
    er = r_of[rows]

    groups = [list(range(gs, min(gs + GROUP, NRB)))
              for gs in range(0, NRB, GROUP)]

    # --- per (core, chunk) pairing -------------------------------------
    core_data = [[None] * NCHUNKS for _ in range(NCORES)]
    core_pos = [[None] * NCHUNKS for _ in range(NCORES)]
    for k in range(NCORES):
        mk = ecore == k
        for c in range(NCHUNKS):
            m = mk & (chunk == c)
            ccol = (cols[m] - c * CHUNK).astype(np.int64)
            crb = erb[m].astype(np.int64)
            crow = er[m].astype(np.int64)
            cval = vals[m]
            pairs, unm, nbrs = _pair_core_chunk(crb, ccol)
            core_data[k][c] = (pairs, unm, crow, cval)
            core_pos[k][c] = _forest_to_positions(nbrs)

    # --- budgets across cores ------------------------------------------
    # per cell, choose BP minimizing BP + max_k(U_k + 2*max(0, P_k - BP));
    # cores with more pairs than BP unpair the excess into unmatched
    BP = np.zeros((NRB, NCHUNKS), np.int64)
    BU = np.zeros((NRB, NCHUNKS), np.int64)
    for c in range(NCHUNKS):
        for rb in range(NRB):
            P = np.array([len(core_data[k][c][0][rb]) for k in range(NCORES)])
            U = np.array([len(core_data[k][c][1][rb]) for k in range(NCORES)])
            cand = np.unique(P)
            best = None
            for bp in cand:
                tot = bp + (U + 2 * np.maximum(0, P - bp)).max()
                if best is None or tot < best[0]:
                    best = (tot, bp)
            bp = int(best[1])
            BP[rb, c] = bp
            BU[rb, c] = int((U + 2 * np.maximum(0, P - bp)).max())
            for k in range(NCORES):
                pairs, unm, crow, cval = core_data[k][c]
                excess = len(pairs[rb]) - bp
                if excess > 0:
                    moved = pairs[rb][bp:]
                    pairs[rb] = pairs[rb][:bp]
                    extra = np.concatenate([moved[:, [0, 1]],
                                            moved[:, [2, 3]]], axis=0)
                    unm[rb] = np.concatenate([unm[rb], extra], axis=0)

    # --- call layout (shared across cores) -----------------------------
    call_spec = []          # (chunk, desc_start, ndesc)
    PO = np.zeros((NRB, NCHUNKS), np.int64)
    UO = np.zeros((NRB, NCHUNKS), np.int64)
    dpos = 0
    for g in groups:
        for c in range(NCHUNKS):
            start = dpos
            for rb in g:
                PO[rb, c] = dpos
                dpos += int(BP[rb, c])
            # unmatched in DESCENDING rb order so the paired->unmatched
            # transition and all internal transitions stay span<=2
            for rb in reversed(g):
                UO[rb, c] = dpos
                dpos += int(BU[rb, c])
            dpos = (dpos + 127) & ~127
            call_spec.append((c, start, dpos - start))
    DT = dpos

    # shared desc -> rb maps (rb or -1 = dead), per half
    descA_rb = np.full(DT, -1, np.int64)
    descB_rb = np.full(DT, -1, np.int64)
    for rb in range(NRB):
        for c in range(NCHUNKS):
            descA_rb[PO[rb, c]:PO[rb, c] + BP[rb, c]] = rb
            descB_rb[PO[rb, c]:PO[rb, c] + BP[rb, c]] = rb
            descA_rb[UO[rb, c]:UO[rb, c] + BU[rb, c]] = rb

    # --- slab plan (shared) --------------------------------------------
    # per call: list of [slot, half, base_rb_local, width, segs]; segs =
    # [(psum_off, n, stop)] split at 512-fp32 PSUM bank boundaries. Lookup
    # arrays slabcol[s, half] (global rowv column) and base[s, half].
    slab_plan = []
    call_lookup = []
    nslab = 0
    call_i = 0
    NBANK = GROUP * 128 // 512
    bank_writes = []        # per group: per bank count of seg matmuls
    for gi, g in enumerate(groups):
        g0 = g[0]
        bw = [0] * NBANK
        for cidx in range(NCHUNKS):
            c, start, ndesc = call_spec[call_i]
            nslots = ndesc // 128
            slabs = []
            scol = np.full((nslots, 2, GROUP), -1, np.int64)
            sbase = np.zeros((nslots, 2, GROUP), np.int64)
            for s in range(nslots):
                dlo, dhi = start + s * 128, start + (s + 1) * 128
                for half, drb in ((0, descA_rb), (1, descB_rb)):
                    live = drb[dlo:dhi]
                    live = live[live >= 0]
                    if live.size == 0:
                        continue
                    lo, hi = int(live.min()), int(live.max())
                    # split spans wider than 4 rbs into <=4-rb sub-slabs
                    r = lo
                    while r <= hi:
                        rhi = min(r + 3, hi)
                        sub = live[(live >= r) & (live <= rhi)]
                        if sub.size:
                            slo, shi = int(sub.min()), int(sub.max())
                            base_local = slo - g0
                            width = (shi - slo + 1) * 128
                            base = base_local * 128
                            segs = []
                            off, end = base, base + width
                            while off < end:
                                seg_end = min(end, (off // 512 + 1) * 512)
                                segs.append([off, seg_end - off, False])
                                bw[off // 512] += 1
                                off = seg_end
                            slabs.append([s, half, base_local, width, segs])
                            for rbx in range(slo, shi + 1):
                                scol[s, half, rbx - g0] = nslab
                                sbase[s, half, rbx - g0] = base_local
                            nslab += 1
                        r = rhi + 1
            slab_plan.append(slabs)
            call_lookup.append((scol, sbase))
            call_i += 1
        bank_writes.append(bw)
    NSLAB = nslab

    # per (group, bank): the last seg matmul gets the PSUM stop flag
    for gi in range(len(groups)):
        seen = [0] * NBANK
        for cidx in range(NCHUNKS):
            ci_ = gi * NCHUNKS + cidx
            for sl in slab_plan[ci_]:
                for seg in sl[4]:
                    b = seg[0] // 512
                    seen[b] += 1
                    seg[2] = bool(seen[b] == bank_writes[gi][b])

    # --- per-core colidx / rowv / valv (vectorized) --------------------
    per_core = []
    for k in range(NCORES):
        colidx = np.zeros(DT, np.int16)
        rowv = np.zeros((128, NSLAB), f16)
        valv = np.zeros((128, NSLAB), f16)
        for gi, g in enumerate(groups):
            g0 = g[0]
            for cidx in range(NCHUNKS):
                call_j = gi * NCHUNKS + cidx
                c, start, ndesc = call_spec[call_j]
                pairs, unm, crow, cval = core_data[k][c]
                pos = core_pos[k][c]
                scol, sbase = call_lookup[call_j]
                for rb in g:
                    pr = pairs[rb]
                    n = len(pr)
                    if n:
                        acol, atok, bcol, btok = pr.T
                        pa, pb = pos[acol].astype(np.int64), \
                            pos[bcol].astype(np.int64)
                        fwd = pa + 1 == pb
                        rev = pb + 1 == pa
                        assert (fwd | rev).all(), "pair not adjacent"
                        d = PO[rb, c] + np.arange(n)
                        colidx[d] = np.where(fwd, pa, pb).astype(np.int16)
                        first = np.where(fwd, atok, btok)
                        second = np.where(fwd, btok, atok)
                        srel = (d - start) // 128
                        p = (d - start) % 128
                        for half, tk in ((0, first), (1, second)):
                            cj = scol[srel, half, rb - g0]
                            bl = sbase[srel, half, rb - g0]
                            assert (cj >= 0).all()
                            rowv[p, cj] = ((rb - g0 - bl) * 128
                                           + crow[tk]).astype(f16)
                            valv[p, cj] = cval[tk].astype(f16)
                    un = unm[rb]
                    n = len(un)
                    if n:
                        acol, atok = un.T
                        d = UO[rb, c] + np.arange(n)
                        colidx[d] = pos[acol].astype(np.int16)
                        srel = (d - start) // 128
                        p = (d - start) % 128
                        cj = scol[srel, 0, rb - g0]
                        bl = sbase[srel, 0, rb - g0]
                        assert (cj >= 0).all()
                        rowv[p, cj] = ((rb - g0 - bl) * 128
                                       + crow[atok]).astype(f16)
                        valv[p, cj] = cval[atok].astype(f16)
        per_core.append(dict(
            colidx=np.ascontiguousarray(
                np.tile(colidx.reshape(DT // 16, 16).T, (8, 1))),
            rowv=np.ascontiguousarray(rowv),
            valv=np.ascontiguousarray(valv),
        ))

    max_call = max(cs[2] for cs in call_spec)
    assert 2 * max_call <= 5120, f"ring overflow risk: max call {max_call}"
    key = (DT, NSLAB)
    meta = dict(groups=groups, call_spec=call_spec, DT=DT, NSLAB=NSLAB,
                slab_plan=slab_plan, bank_writes=bank_writes,
                per_core=per_core, node_of=node_of,
                core_pos=core_pos, key=key)
    return key, meta


# ---------------------------------------------------------------------------
# device kernel
# ---------------------------------------------------------------------------

def build_kernel(meta):
    groups, call_spec = meta["groups"], meta["call_spec"]
    DT, NSLAB = meta["DT"], meta["NSLAB"]
    slab_plan = meta["slab_plan"]
    bank_writes = meta["bank_writes"]
    NBANK = GROUP * 128 // 512

    nc = bacc.Bacc("TRN2", target_bir_lowering=False, debug=False,
                   num_devices=NCORES, num_swdge_queues=4,
                   dynamic_dma_scratch_size=81920)
    table = nc.dram_tensor("table", [NCHUNKS, CHUNKP, D], dt.float16,
                           kind="ExternalInput")
    w = nc.dram_tensor("w", [D, D], dt.float16, kind="ExternalInput")
    colidx = nc.dram_tensor("colidx", [128, DT // 16], dt.int16,
                            kind="ExternalInput")
    rowv = nc.dram_tensor("rowv", [128, NSLAB], dt.float16,
                          kind="ExternalInput")
    valv = nc.dram_tensor("valv", [128, NSLAB], dt.float16,
                          kind="ExternalInput")
    hout = nc.dram_tensor("hout", [128, NRB, D], dt.float16,
                          kind="ExternalOutput")
    iota_np = np.tile(np.arange(512, dtype=np.float32)[None, :], (128, 1))
    iota = nc.inline_tensor(iota_np.astype(f16), "iota")

    with tile.TileContext(nc) as tc, ExitStack() as ctx:
        cpool = ctx.enter_context(tc.tile_pool(name="c", bufs=1))
        mpool = ctx.enter_context(tc.tile_pool(name="m", bufs=2))
        spool = ctx.enter_context(tc.tile_pool(name="s", bufs=1))
        zspool = ctx.enter_context(tc.tile_pool(name="zs", bufs=2))
        hpool = ctx.enter_context(tc.tile_pool(name="h", bufs=2))
        pspool = ctx.enter_context(
            tc.tile_pool(name="ps", bufs=1, space="PSUM"))
        cipool = ctx.enter_context(tc.tile_pool(name="ci", bufs=2))

        it = cpool.tile([128, 512], dt.float16)
        nc.sync.dma_start(it[:], iota[:])
        wt = cpool.tile([128, 128], dt.float16)
        nc.sync.dma_start(wt[:], w[:])
        rvh = cpool.tile([128, NSLAB], dt.float16)
        nc.sync.dma_start(rvh[:], rowv[:])
        vvh = cpool.tile([128, NSLAB], dt.float16)
        nc.sync.dma_start(vvh[:], valv[:])
        rv = cpool.tile([128, NSLAB], dt.float32)
        nc.scalar.copy(rv[:], rvh[:])
        vv = cpool.tile([128, NSLAB], dt.float32)
        nc.scalar.copy(vv[:], vvh[:])
        st0 = cpool.tile([128, 512], dt.float16)
        nc.gpsimd.memset(st0[:], 0.0)

        call_i = 0
        slab_base = 0
        for gi, g in enumerate(groups):
            ng = len(g)
            gstart = call_spec[gi * NCHUNKS][1]
            gend = (call_spec[gi * NCHUNKS + NCHUNKS - 1][1]
                    + call_spec[gi * NCHUNKS + NCHUNKS - 1][2])
            ci = cipool.tile([128, (gend - gstart) // 16], dt.int16,
                             tag="ci", name="ci")
            nc.sync.dma_start(ci[:], colidx[:, gstart // 16:gend // 16])
            zp = pspool.tile([128, GROUP * 128], dt.float32,
                             tag=f"z{gi % 2}", name="zp")
            for b in range(NBANK):
                nc.tensor.matmul(zp[:, b * 512:(b + 1) * 512],
                                 it[:, 0:128], st0[:],
                                 start=True,
                                 stop=bool(bank_writes[gi][b] == 0))
            hg = hpool.tile([128, GROUP, 128], dt.float16, tag="hg",
                            name="hg")
            mts = []
            for cidx in range(NCHUNKS):
                c, start, ndesc = call_spec[call_i + cidx]
                nslots = ndesc // 128
                mt = mpool.tile([128, nslots, 256], dt.float16,
                                tag=f"m{c}", name=f"mt{c}")
                in_ap = table[c].copy()
                in_ap.ap[0] = [128, CHUNKP - 1]
                in_ap.ap[1] = [1, 256]
                nc.gpsimd.dma_gather(
                    mt[:], in_ap,
                    ci[:, (start - gstart) // 16:(start - gstart + ndesc) // 16],
                    ndesc, ndesc, 256, elem_step=128,
                    queue_num=cidx, single_packet=False)
                mts.append(mt)
            for cidx in range(NCHUNKS):
                c, start, ndesc = call_spec[call_i]
                slabs = slab_plan[call_i]
                mt = mts[cidx]
                if slabs:
                    aslots = sum(sl[3] // 128 for sl in slabs)
                    sa = spool.tile([128, aslots, 128], dt.float16,
                                    tag=f"s{c % 2}", name=f"sa{c}")
                    ao = 0
                    noff = 0
                    for si, (s, half, base_local, width, segs) in \
                            enumerate(slabs):
                        col = slab_base + si
                        # offload ~1/7 of narrow builds to GPSIMD
                        if width == 128 and noff % 7 == 3:
                            eng = nc.gpsimd
                        else:
                            eng = nc.vector
                        if width == 128:
                            noff += 1
                        eng.tensor_scalar(
                            sa[:, ao:ao + width // 128, :], it[:, 0:width],
                            rv[:, col:col + 1], vv[:, col:col + 1],
                            mybir.AluOpType.is_equal, mybir.AluOpType.mult)
                        base = base_local * 128
                        for (off, n, stopf) in segs:
                            nc.tensor.matmul(
                                zp[:, off:off + n],
                                mt[:, s, half * 128:half * 128 + 128],
                                sa[:, ao + (off - base) // 128:
                                   ao + (off - base) // 128 + n // 128, :],
                                start=False, stop=bool(stopf))
                        ao += width // 128
                slab_base += len(slabs)
                call_i += 1
            zs = zspool.tile([128, GROUP * 128], dt.float16, name="zs")
            nc.scalar.copy(zs[:], zp[:])
            for j in range(ng):
                yp = pspool.tile([128, 128], dt.float32, tag=f"y{j % 2}",
                                 name="yp")
                nc.tensor.matmul(yp[:], zs[:, j * 128:(j + 1) * 128], wt[:],
                                 start=True, stop=True)
                nc.scalar.activation(hg[:, j, :], yp[:],
                                     mybir.ActivationFunctionType.Relu)
            nc.scalar.dma_start(hout[:, g[0]:g[0] + ng, :], hg[:, :ng, :])

    nc.compile()
    return nc


_NC_CACHE = {}
_PREP_CACHE = {}


def _get_nc(meta):
    key = meta["key"]
    if key not in _NC_CACHE:
        _NC_CACHE[key] = build_kernel(meta)
    return _NC_CACHE[key]


def _digest(*arrs):
    h = hashlib.blake2b(digest_size=16)
    for a in arrs:
        h.update(np.ascontiguousarray(a).tobytes())
    return h.hexdigest()


def _build_tables(hfull_f16, meta):
    """Per-core permuted chunk tables [NCHUNKS, CHUNKP, D] f16."""
    tables = []
    for k in range(NCORES):
        tab = np.zeros((NCHUNKS, CHUNKP, D), f16)
        for c in range(NCHUNKS):
            pos = meta["core_pos"][k][c]          # pos[col_local]
            inv = np.empty(CHUNK, np.int64)
            inv[pos] = np.arange(CHUNK)
            tab[c, :CHUNK] = hfull_f16[c * CHUNK + inv]
        tables.append(tab)
    return tables


def _run_layer(nc, tables, w_f32, meta, trace=False):
    in_maps = [
        dict(table=tables[k], w=w_f32.astype(f16),
             colidx=pc["colidx"], rowv=pc["rowv"], valv=pc["valv"])
        for k, pc in enumerate(meta["per_core"])
    ]
    res = run_bass_kernel_spmd(nc, in_maps, list(range(NCORES)), trace=trace)
    node_of = meta["node_of"]
    hfull = np.zeros((N_NODES, D), f16)
    for k in range(NCORES):
        hk = np.asarray(res.results[k]["hout"])
        flat = hk.transpose(1, 0, 2).reshape(RPC, D)
        nid = node_of[k].reshape(RPC)
        m = nid >= 0
        hfull[nid[m]] = flat[m]
    return hfull, res


def kernel(X_mask, adj_rows, adj_cols, adj_vals, W1, W2):
    global LAST_EXEC_NS
    dig = _digest(adj_rows, adj_cols, adj_vals)
    if dig not in _PREP_CACHE:
        _PREP_CACHE[dig] = prep_edges(adj_rows, adj_cols, adj_vals)
    key, meta = _PREP_CACHE[dig]
    nc = _get_nc(meta)

    x16 = np.ascontiguousarray(np.asarray(X_mask, np.float32)).astype(f16)
    t1 = _build_tables(x16, meta)
    h1, res1 = _run_layer(nc, t1, np.asarray(W1, np.float32), meta)
    t2 = _build_tables(h1, meta)
    out, res2 = _run_layer(nc, t2, np.asarray(W2, np.float32), meta)

    ns = [r.exec_time_ns for r in (res1, res2)]
    LAST_EXEC_NS = sum(n for n in ns if n) if any(ns) else None
    return out.astype(np.float32)
